# revision 21
# baseline (speedup 1.0000x reference)
"""Trainium2 Bass kernel for the moe_routing classifier problem (v4).

Computation (per batch row b, class c):
  cos[b,c,s]  = cosine(emb[b], weight[c,s])            (64 sub-prototypes)
  top-8 over s, softmax weights w, protos = sum_k w_k * weight[c, idx_k]
  out[b,c]    = ((1 + cosine(protos, emb[b])) / 2 + 1e-8) / 0.1

Scale-invariant form (any per-(b,c) scale of E cancels):
  u        = cos/2
  Ebar_s   = ||w_s|| * exp(u_s) * mask(u_s >= thr8)
  NUM      = 2 * sum_s u_s Ebar_s          (= e.protos/||e|| up to scale)
  DEN^2    = Ebar^T Ghat Ebar              (Ghat = normalized-anchor Gram)
  out      = 5 * NUM / sqrt(DEN^2) + 5 + 1e-7

Pipeline per core (32 classes = 16 pairs, full B):
  A) b-major scores (PE) -> DMA evac -> max8 per class (DVE) -> thr8,
     thr transposed+replicated via DMA into per-pair s-major tiles.
  B) s-major, per (pair q, b-half): psum chain on PE:
       u = VhatT_q x ehatT          (bf16 matmul)
       u += I15 x thrArep15_q       (w' = u - thr + delta)
       T = tanh(K*w')  [scalar]     (soft mask, +-1 saturated)
       u += I15 x T                 (pval = w' + 15T)
     Ebar = exp(pval + lnw - 15) [scalar, per-partition bias]
     P0 = pval * Ebar [DVE from psum]; H = Ghat_q x Ebar [PE];
     P1 = Ebar * H [gpsimd, H DMA-evacuated]
     selector matmul accumulates [sum P0 | sum P1 | sum Ebar] over q.
  Finale: NUM = 2*S0 + 2*(thr-delta-15)*SE, out = 10*(...)*rsqrt(S1)+bias.
"""

import numpy as np

B, D, C, S = 1024, 128, 256, 64
NCORES = 8
C_LOC = C // NCORES        # 32 classes per core
CS = C_LOC * S             # 2048 anchor rows per core
P = 128
NBT = B // P               # 8 batch tiles
NQ = C_LOC // 2            # 16 class pairs
K_SOFT = 1.0e6             # tanh cliff sharpness
DELTA = 1.0e-5             # threshold down-shift keeping the 8th selected
BIGM = 15.0
H_FD = 0.01
OUT_BIAS = 5.0 + 1e-7

_CACHE = {}
DEBUG = False


def build_nc():
    import concourse.bass as bass
    import concourse.tile as tile
    from concourse import bacc, mybir
    from concourse.masks import make_identity
    from contextlib import ExitStack

    f32 = mybir.dt.float32
    f32r = mybir.dt.float32r
    bf16 = mybir.dt.bfloat16
    AF = mybir.ActivationFunctionType
    ALU = mybir.AluOpType

    nc = bacc.Bacc(None, target_bir_lowering=False)
    emb_d = nc.dram_tensor("emb", [B, D], f32, kind="ExternalInput")
    w_d = nc.dram_tensor("weight", [CS, D], f32, kind="ExternalInput")
    out_d = nc.dram_tensor("out", [B, C_LOC], f32, kind="ExternalOutput")
    dbg_d = {}
    if DEBUG:
        dbg_d["dbg_usb"] = nc.dram_tensor("dbg_usb", [P, CS], f32, kind="ExternalOutput")
        dbg_d["dbg_mx8"] = nc.dram_tensor("dbg_mx8", [P, C_LOC * 8], f32, kind="ExternalOutput")
        dbg_d["dbg_thrA"] = nc.dram_tensor("dbg_thrA", [C_LOC, B], f32, kind="ExternalOutput")
        dbg_d["dbg_rhs"] = nc.dram_tensor("dbg_rhs", [P, 3 * 512], f32, kind="ExternalOutput")
        dbg_d["dbg_fin"] = nc.dram_tensor("dbg_fin", [C_LOC, 3 * 512], f32, kind="ExternalOutput")
        dbg_d["dbg_T"] = nc.dram_tensor("dbg_T", [P, 512], f32, kind="ExternalOutput")
        dbg_d["dbg_pval"] = nc.dram_tensor("dbg_pval", [P, 512], f32, kind="ExternalOutput")

    with tile.TileContext(nc) as tc, ExitStack() as ctx:
        sing = ctx.enter_context(tc.tile_pool(name="sing", bufs=1))
        work = ctx.enter_context(tc.tile_pool(name="work", bufs=3))
        ps_a = ctx.enter_context(tc.tile_pool(name="ps_a", bufs=1, space="PSUM"))
        ps_ch = ctx.enter_context(tc.tile_pool(name="ps_ch", bufs=1, space="PSUM"))
        ps_h = ctx.enter_context(tc.tile_pool(name="ps_h", bufs=1, space="PSUM"))
        ps_sel = ctx.enter_context(tc.tile_pool(name="ps_sel", bufs=1, space="PSUM"))

        identf = sing.tile([P, P], f32)
        make_identity(nc, identf[:])
        ident15 = sing.tile([P, P], bf16)
        nc.vector.tensor_scalar(ident15[:], identf[:], BIGM, None, ALU.mult)

        # ---------------- load inputs ----------------
        En = sing.tile([P, NBT, D], f32)
        nc.sync.dma_start(En[:], emb_d[:].rearrange("(t p) d -> p t d", p=P))
        Wn = sing.tile([P, NQ, D], f32)
        nc.gpsimd.dma_start(Wn[:], w_d[:].rearrange("(t p) d -> p t d", p=P))

        # ---------------- emb preproc: esT = emb^T / (2||e||), bf16 ------
        esqt = sing.tile([P, B], f32)
        nc.vector.tensor_mul(esqt[:], En[:].rearrange("p t d -> p (t d)"),
                             En[:].rearrange("p t d -> p (t d)"))
        nesq = sing.tile([P, NBT], f32)
        nc.vector.tensor_reduce(nesq[:], esqt[:].rearrange("p (t d) -> p t d", d=D),
                                axis=mybir.AxisListType.X, op=ALU.add)
        # table set 1: sqrt -> ne2 = sqrt(4*|e|^2) = 2||e||, ihe = 1/(2||e||)
        ne2 = sing.tile([P, NBT], f32)
        nc.scalar.activation(ne2[:], nesq[:], AF.Sqrt, scale=4.0)
        ihe = sing.tile([P, NBT], f32)
        ihe_s = sing.tile([P, NBT], f32)
        nc.vector.reciprocal_approx_accurate(ihe[:], ne2[:], ihe_s[:])
        EnS = sing.tile([P, NBT, D], f32)
        ihe_b = ihe[:, :, None].to_broadcast([P, NBT, D])
        nc.vector.tensor_mul(EnS[:], En[:], ihe_b)
        esT = sing.tile([P, B], bf16)
        for g in range(2):
            pst = ps_ch.tile([P, 4 * P], f32, tag="ch1")
            for t in range(4):
                nc.tensor.transpose(pst[:, t * P:(t + 1) * P], EnS[:, 4 * g + t],
                                    identf[:])
            nc.scalar.copy(esT[:, g * 512:(g + 1) * 512], pst[:])

        # ---------------- weight preproc ----------------
        wsqt = sing.tile([P, CS], f32)
        nc.vector.tensor_mul(wsqt[:], Wn[:].rearrange("p t d -> p (t d)"),
                             Wn[:].rearrange("p t d -> p (t d)"))
        nwsq = sing.tile([P, NQ], f32)
        nc.vector.tensor_reduce(nwsq[:], wsqt[:].rearrange("p (t d) -> p t d", d=D),
                                axis=mybir.AxisListType.X, op=ALU.add)
        nwt = sing.tile([P, NQ], f32)
        nc.scalar.activation(nwt[:], nwsq[:], AF.Sqrt)
        inw = sing.tile([P, NQ], f32)
        inw_s = sing.tile([P, NQ], f32)
        nc.vector.reciprocal_approx_accurate(inw[:], nwt[:], inw_s[:])
        # table set 2: natural_log -> lnw - 15 = 0.5*ln(|w|^2) - 15
        lnwsq = sing.tile([P, NQ], f32)
        nc.scalar.activation(lnwsq[:], nwsq[:], AF.Ln)
        lnw0 = sing.tile([P, NQ], f32)
        nc.vector.tensor_scalar(lnw0[:], lnwsq[:], 0.5, None, ALU.mult)

        Vn = sing.tile([P, NQ, D], f32)
        inw_b = inw[:, :, None].to_broadcast([P, NQ, D])
        nc.vector.tensor_mul(Vn[:], Wn[:], inw_b)
        VT = sing.tile([P, CS], bf16)
        for g in range(4):
            pst = ps_ch.tile([P, 4 * P], f32, tag="ch1")
            for t in range(4):
                nc.tensor.transpose(pst[:, t * P:(t + 1) * P], Vn[:, 4 * g + t],
                                    identf[:])
            nc.scalar.copy(VT[:, g * 512:(g + 1) * 512], pst[:])

        # normalized Gram, block-diagonal per pair: GP [128, q, 128] bf16
        Gt = sing.tile([S, CS], bf16)
        for g in range(4):
            psg = ps_h.tile([P, 512], f32, tag="h")
            for i in range(8):
                cc = 8 * g + i
                csl = slice(cc * S, (cc + 1) * S)
                nc.tensor.matmul(psg[0:S, i * S:(i + 1) * S], VT[:, csl], VT[:, csl])
            nc.scalar.copy(Gt[:, g * 512:(g + 1) * 512], psg[0:S, :])
        GP = sing.tile([P, NQ, P], bf16)
        nc.vector.memset(GP[:], 0.0)
        gt3 = Gt[:].rearrange("p (q j) -> p q j", j=2 * S)
        nc.sync.dma_start(GP[:][0:S, :, 0:S], gt3[:, :, 0:S])
        nc.sync.dma_start(GP[:][S:2 * S, :, S:2 * S], gt3[:, :, S:2 * S])

        # selector stationaries: selq [128, NQ, 32] bf16
        selq = sing.tile([P, NQ, C_LOC], bf16)
        nc.vector.memset(selq[:], 0.0)
        for q in range(NQ):
            for par in range(2):
                nc.vector.memset(
                    selq[:][par * S:(par + 1) * S, q, 2 * q + par:2 * q + par + 1],
                    1.0)
        # threshold replication stationaries: band matrix [32, 2048] bf16,
        # row c = 15 on cols [64c, 64c+64); slice [:, q*128:(q+1)*128] per q.
        repm = sing.tile([C_LOC, CS], bf16)
        nc.gpsimd.memset(repm[:], 0.0)
        nc.gpsimd.affine_select(
            out=repm[:].rearrange("c (j r) -> c j r", r=S),
            in_=repm[:].rearrange("c (j r) -> c j r", r=S),
            compare_op=ALU.not_equal,
            fill=BIGM,
            base=0,
            pattern=[[-1, C_LOC], [0, S]],
            channel_multiplier=1,
        )

        # table set 3 (main): exp_and_others (exp, tanh, copy)
        texp = sing.tile([P, 1], f32)
        nc.vector.memset(texp[:], 1.0)
        nc.scalar.activation(texp[:], texp[:], AF.Exp)
        nc.scalar.activation(texp[:], texp[:], AF.Tanh)

        # ---------------- phase A: b-major scores, max8, thresholds ------
        u_sb = {}
        mx8 = {}
        thrPh = {}
        dmae = [nc.sync, nc.scalar, nc.gpsimd, nc.sync]

        def phase_a_bt(bt):
            u_t = work.tile([P, CS], f32, tag="usb", bufs=3)
            u_sb[bt] = u_t
            m_t = work.tile([P, C_LOC * 8], f32, tag="mx8", bufs=2)
            mx8[bt] = m_t
            for ch in range(4):
                pps = ps_a.tile([P, 512], f32, tag="bmaj")
                nc.tensor.matmul(pps[:], esT[:, bt * P:(bt + 1) * P],
                                 VT[:, ch * 512:(ch + 1) * 512])
                if ch % 2 == 0:
                    nc.scalar.copy(u_t[:, ch * 512:(ch + 1) * 512], pps[:])
                else:
                    nc.vector.tensor_copy(u_t[:, ch * 512:(ch + 1) * 512], pps[:])
            for c in range(C_LOC):
                nc.vector.max(m_t[:, c * 8:(c + 1) * 8], u_t[:, c * S:(c + 1) * S])
            if DEBUG and bt == 0:
                nc.sync.dma_start(dbg_d["dbg_usb"][:], u_t[:])
                nc.sync.dma_start(dbg_d["dbg_mx8"][:], m_t[:])
            thr_bt = work.tile([P, C_LOC], f32, tag="thr_b", bufs=2)
            nc.vector.tensor_copy(
                thr_bt[:], m_t[:].rearrange("p (c k) -> p c k", k=8)[:, :, 7])
            if bt % 4 == 0:
                thr_t = ps_sel.tile([C_LOC, 512], f32, tag="thr")
                thrPh[bt // 4] = thr_t
            nc.tensor.transpose(thrPh[bt // 4][:, (bt % 4) * P:(bt % 4 + 1) * P],
                                thr_bt[:], identf[:])

        # thrA/thrB in s-major small domain; per-pair replication matmul
        # stationary RepM_q[c, p] = 15 if class_q(p) == c broadcasts them.
        thrA15 = sing.tile([C_LOC, B], f32)    # (delta - thr)/15
        thrAhi = sing.tile([C_LOC, B], bf16)   # bf16 head of thrA15
        thrAlo = sing.tile([C_LOC, B], bf16)   # bf16 tail of thrA15
        thrChi = sing.tile([C_LOC, B], bf16)   # bf16((delta - thr - 15)/15)
        thrB = sing.tile([C_LOC, B], f32)      # -15*Chi - 15 = thr_eff - delta

        def thr_finalize(half):
            bsl = slice(half * 512, (half + 1) * 512)
            nc.scalar.activation(thrA15[:, bsl], thrPh[half][:], AF.Copy,
                                 scale=-1.0 / BIGM, bias=DELTA / BIGM)
            tb = work.tile([C_LOC, 512], f32, tag="tb", bufs=2)
            nc.scalar.activation(tb[:], thrPh[half][:], AF.Copy,
                                 scale=-1.0 / BIGM, bias=DELTA / BIGM - 1.0)
            nc.vector.tensor_copy(thrChi[:, bsl], tb[:])
            nc.vector.tensor_scalar(thrB[:, bsl], thrChi[:, bsl], -BIGM, -BIGM,
                                    ALU.mult, ALU.add)
            nc.vector.tensor_copy(thrAhi[:, bsl], thrA15[:, bsl])
            tlo = work.tile([C_LOC, 512], f32, tag="tlo", bufs=2)
            nc.vector.tensor_sub(tlo[:], thrA15[:, bsl], thrAhi[:, bsl])
            nc.vector.tensor_copy(thrAlo[:, bsl], tlo[:])

        # ---------------- phase B unit ----------------
        selPs = {}

        def phase_b_unit(q, half):
            bsl = slice(half * 512, (half + 1) * 512)
            ch1 = ps_ch.tile([P, 512], f32, tag="ch1")
            nc.tensor.matmul(ch1[:], VT[:, q * P:(q + 1) * P], esT[:, bsl],
                             start=True, stop=False)
            nc.tensor.matmul(ch1[:], repm[:, q * P:(q + 1) * P],
                             thrAhi[:, bsl], start=False, stop=False)
            nc.tensor.matmul(ch1[:], repm[:, q * P:(q + 1) * P],
                             thrAlo[:, bsl], start=False, stop=True)
            T_t = work.tile([P, 512], bf16, tag="tanh", bufs=3)
            nc.scalar.activation(T_t[:], ch1[:], AF.Tanh, scale=K_SOFT)
            chain = ps_ch.tile([P, 512], f32, tag="ch2")
            nc.tensor.matmul(chain[:], VT[:, q * P:(q + 1) * P], esT[:, bsl],
                             start=True, stop=False)
            nc.tensor.matmul(chain[:], repm[:, q * P:(q + 1) * P],
                             thrChi[:, bsl], start=False, stop=False)
            nc.tensor.matmul(chain[:], ident15[:], T_t[:], start=False,
                             stop=True)
            rhs = work.tile([P, 3 * 512], bf16, tag="rhs", bufs=3)
            Eb = rhs[:, 1024:1536]
            nc.scalar.activation(Eb, chain[:], AF.Exp, bias=lnw0[:, q:q + 1])
            nc.vector.tensor_mul(rhs[:, 0:512], chain[:], Eb)       # P0

            hps = ps_h.tile([P, 512], f32, tag="h")
            nc.tensor.matmul(hps[:], GP[:, q, :], Eb)
            nc.vector.tensor_mul(rhs[:, 512:1024], hps[:], Eb)      # P1
            if DEBUG and q == 0 and half == 0:
                Tf = sing.tile([P, 512], f32, tag="Tf")
                nc.vector.tensor_copy(Tf[:], T_t[:])
                nc.sync.dma_start(dbg_d["dbg_T"][:], Tf[:])
                pvf = sing.tile([P, 512], f32, tag="pvf")
                nc.vector.tensor_copy(pvf[:], chain[:])
                nc.sync.dma_start(dbg_d["dbg_pval"][:], pvf[:])
                rhsf = sing.tile([P, 3 * 512], f32, tag="rhsf")
                nc.vector.tensor_copy(rhsf[:], rhs[:])
                nc.sync.dma_start(dbg_d["dbg_rhs"][:], rhsf[:])
                thrAf = sing.tile([C_LOC, B], f32, tag="thrAf")
                nc.vector.memset(thrAf[:], 0.0)
                nc.vector.tensor_copy(thrAf[:, 0:512], thrA15[:, 0:512])
                nc.sync.dma_start(dbg_d["dbg_thrA"][:], thrAf[:])
            for blk in range(3):
                nc.tensor.matmul(selPs[half][:, blk * 512:(blk + 1) * 512],
                                 selq[:, q, :], rhs[:, blk * 512:(blk + 1) * 512],
                                 start=(q == 0), stop=(q == NQ - 1))

        # ---------------- finale per half ----------------
        osb = sing.tile([P, NBT, C_LOC], f32)

        def finale(half):
            fin = sing.tile([C_LOC, 3 * 512], f32, tag=f"fin{half}")
            nc.scalar.copy(fin[:], selPs[half][:])
            if DEBUG and half == 0:
                nc.sync.dma_start(dbg_d["dbg_fin"][:], fin[:])
            SEh = fin[:, 0:512]
            S1 = fin[:, 512:1024]
            SE = fin[:, 1024:1536]
            bsl = slice(half * 512, (half + 1) * 512)
            t1 = sing.tile([C_LOC, 512], f32, tag=f"t1{half}")
            nc.vector.tensor_mul(t1[:], SE, thrB[:, bsl])
            t2 = sing.tile([C_LOC, 512], f32, tag=f"t2{half}")
            nc.vector.tensor_add(t2[:], SEh, t1[:])
            return t2, S1

        # ---------------- schedule ----------------
        # A for first half of batch tiles, finalize thr half 0, then
        # interleave B(half 0) with A(bt 4..7), etc.
        for bt in range(4):
            phase_a_bt(bt)
        thr_finalize(0)
        sel0_t = ps_sel.tile([C_LOC, 3 * 512], f32, tag="sel")
        selPs[0] = sel0_t
        for q in range(NQ):
            if q < 4:
                phase_a_bt(4 + q)
            if q == 4:
                thr_finalize(1)
            phase_b_unit(q, 0)
        num0, S1h0 = finale(0)
        sel1_t = ps_sel.tile([C_LOC, 3 * 512], f32, tag="sel")
        selPs[1] = sel1_t
        for q in range(NQ):
            phase_b_unit(q, 1)
        num1, S1h1 = finale(1)

        # table set 4: sqrt for the finale, reciprocal on DVE
        sq0 = sing.tile([C_LOC, 512], f32, tag="sq0")
        sq1 = sing.tile([C_LOC, 512], f32, tag="sq1")
        nc.scalar.activation(sq0[:], S1h0, AF.Sqrt)
        nc.scalar.activation(sq1[:], S1h1, AF.Sqrt)
        rsq0 = sing.tile([C_LOC, 512], f32, tag="rsq0")
        rsq1 = sing.tile([C_LOC, 512], f32, tag="rsq1")
        rsq_s = sing.tile([C_LOC, 512], f32, tag="rsqs")
        nc.vector.reciprocal_approx_accurate(rsq0[:], sq0[:], rsq_s[:])
        nc.vector.reciprocal_approx_accurate(rsq1[:], sq1[:], rsq_s[:])
        ofin = sing.tile([C_LOC, B], f32)
        nc.vector.tensor_mul(ofin[:, 0:512], num0[:], rsq0[:])
        nc.vector.tensor_mul(ofin[:, 512:1024], num1[:], rsq1[:])
        # out = 10 * ofin + OUT_BIAS, transposed to b-major
        for g in range(2):
            pst = ps_ch.tile([P, 512], f32, tag="ch1")
            for t in range(4):
                bt = 4 * g + t
                nc.tensor.transpose(pst[:, t * C_LOC:(t + 1) * C_LOC],
                                    ofin[:, bt * P:(bt + 1) * P],
                                    identf[0:C_LOC, 0:C_LOC])
            nc.scalar.activation(
                osb[:, 4 * g:4 * g + 4, :].rearrange("p t c -> p (t c)"),
                pst[:, 0:4 * C_LOC], AF.Copy, scale=10.0, bias=OUT_BIAS)
        nc.sync.dma_start(out_d[:].rearrange("(t p) c -> p t c", p=P), osb[:])

    nc.compile()
    return nc


def _get_nc():
    if "nc" not in _CACHE:
        _CACHE["nc"] = build_nc()
    return _CACHE["nc"]


def kernel(emb: np.ndarray, weight: np.ndarray) -> np.ndarray:
    from concourse.bass_utils import run_bass_kernel_spmd

    emb = np.ascontiguousarray(np.asarray(emb, dtype=np.float32))
    weight = np.ascontiguousarray(np.asarray(weight, dtype=np.float32))
    assert emb.shape == (B, D) and weight.shape == (C, S, D)

    nc = _get_nc()
    in_maps = [
        {
            "emb": emb,
            "weight": np.ascontiguousarray(
                weight[i * C_LOC:(i + 1) * C_LOC].reshape(CS, D)
            ),
        }
        for i in range(NCORES)
    ]
    res = run_bass_kernel_spmd(nc, in_maps, core_ids=list(range(NCORES)))
    return np.concatenate(
        [res.results[i]["out"] for i in range(NCORES)], axis=1
    )


# revision 22
# speedup vs baseline: 1.5439x; 1.5439x over previous
"""Trainium2 Bass kernel for the moe_routing classifier problem (v2).

Computation (per batch row b, class c):
  cos[b,c,s]  = cosine(emb[b], weight[c,s])            (64 sub-prototypes)
  top-8 over s, softmax weights w, protos = sum_k w_k * weight[c, idx_k]
  out[b,c]    = ((1 + cosine(protos, emb[b])) / 2 + 1e-8) / 0.1

Key algebra (per (b,c); E = masked exp of scores, any per-(b,c) scale of E
cancels between numerator and denominator):
  u          = 0.5/||emb_b|| * dotn + BIG   (dotn = emb . What, normalized W)
  x          = u * 1[u >= thr8]             (thr8 = 8th largest u per class)
  Et         = exp(x^T - BIG)               (unselected -> e^-BIG ~ 1e-13)
  h*d2z      = sum_s ||w_s|| * Et * (x^T - BIG)
  np2z       = Et^T G Et  via pair-block-diag Gram matmuls
  out        = 10 * (h*d2z) / sqrt(np2z) + 5 + 1e-7   (all ||emb|| cancel)

Both reductions over s run on the PE array: one accumulating matmul per
class-pair against a [128, 64] selector whose cols 0:32 hold ||w_s||*onehot
(for d2z) and cols 32:64 hold onehot (for np2z).

Sharding: classes split across 8 cores (32 each), emb replicated; each core
writes a [1024, 32] slice of the output.
"""

import numpy as np

B, D, C, S = 1024, 128, 256, 64
NCORES = 8
C_LOC = C // NCORES        # 32 classes per core
CS = C_LOC * S             # 2048 anchor rows per core
P = 128                    # partitions
NBT = B // P               # 8 batch tiles
NWT = CS // P              # 16 weight tiles (= class pairs)
NPAIR = NWT
BIG = 30.0                 # separation constant for masked exp
OUT_BIAS = 5.0 + 1e-7      # ((1+x)/2 + 1e-8) / 0.1 = 5x + 5 + 1e-7

_CACHE = {}


def build_nc():
    import concourse.bass as bass
    import concourse.tile as tile
    from concourse import bacc, mybir
    from concourse.masks import make_identity
    from contextlib import ExitStack

    f32 = mybir.dt.float32
    f32r = mybir.dt.float32r
    bf16 = mybir.dt.bfloat16
    AF = mybir.ActivationFunctionType
    ALU = mybir.AluOpType

    nc = bacc.Bacc(None, target_bir_lowering=False)
    emb_d = nc.dram_tensor("emb", [B, D], f32, kind="ExternalInput")
    w_d = nc.dram_tensor("weight", [CS, D], f32, kind="ExternalInput")
    out_d = nc.dram_tensor("out", [B, C_LOC], f32, kind="ExternalOutput")

    with tile.TileContext(nc) as tc, ExitStack() as ctx:
        sing = ctx.enter_context(tc.tile_pool(name="sing", bufs=1))
        work = ctx.enter_context(tc.tile_pool(name="work", bufs=2))
        small = ctx.enter_context(tc.tile_pool(name="small", bufs=2))
        ps_a = ctx.enter_context(tc.tile_pool(name="ps_a", bufs=1, space="PSUM"))
        ps_tr = ctx.enter_context(tc.tile_pool(name="ps_tr", bufs=2, space="PSUM"))
        ps_h = ctx.enter_context(tc.tile_pool(name="ps_h", bufs=1, space="PSUM"))
        ps_sel = ctx.enter_context(tc.tile_pool(name="ps_sel", bufs=1, space="PSUM"))

        ident = sing.tile([P, P], f32)
        make_identity(nc, ident[:])
        negbig = sing.tile([P, 1], f32)
        nc.vector.memset(negbig[:], -BIG)
        bigt = sing.tile([P, 1], f32)
        nc.vector.memset(bigt[:], BIG)

        # ---------------- load inputs (emb first, separate DMA queues) ----
        En = sing.tile([P, NBT, D], f32)
        nc.sync.dma_start(En[:], emb_d[:].rearrange("(t p) d -> p t d", p=P))
        Wn = sing.tile([P, NWT, D], f32)
        nc.gpsimd.dma_start(Wn[:], w_d[:].rearrange("(t p) d -> p t d", p=P))

        # ---------------- emb-side preproc ----------------
        # squares on DVE (mult+reduce) to keep the scalar engine free
        esqt = sing.tile([P, B], f32)
        nc.vector.tensor_mul(esqt[:], En[:].rearrange("p t d -> p (t d)"),
                             En[:].rearrange("p t d -> p (t d)"))
        esq = sing.tile([P, NBT], f32)
        nc.vector.tensor_reduce(esq[:], esqt[:].rearrange("p (t d) -> p t d", d=D),
                                axis=mybir.AxisListType.X, op=ALU.add)
        ne = sing.tile([P, NBT], f32)          # ||emb||  (sqrt table set)
        nc.scalar.activation(ne[:], esq[:], AF.Sqrt)
        ine = sing.tile([P, NBT], f32)
        hine = sing.tile([P, NBT], f32)        # 0.5/||emb||
        nc.vector.reciprocal_approx_accurate(ine[:], ne[:], hine[:])
        nc.vector.tensor_scalar_mul(hine[:], ine[:], 0.5)

        embT = sing.tile([P, B], f32r)         # emb^T [d, b]
        for g in range(2):
            pst = ps_tr.tile([P, 4 * P], f32, tag="tr")
            for t in range(4):
                nc.tensor.transpose(pst[:, t * P : (t + 1) * P],
                                    En[:, 4 * g + t], ident[:])
            nc.scalar.copy(embT[:, g * 512 : (g + 1) * 512], pst[:])

        # ---------------- weight-side preproc ----------------
        wsqt = sing.tile([P, CS], f32)
        nc.vector.tensor_mul(wsqt[:], Wn[:].rearrange("p t d -> p (t d)"),
                             Wn[:].rearrange("p t d -> p (t d)"))
        nwsq = sing.tile([P, NWT], f32)        # ||w_row||^2 row-tiled
        nc.vector.tensor_reduce(nwsq[:], wsqt[:].rearrange("p (t d) -> p t d", d=D),
                                axis=mybir.AxisListType.X, op=ALU.add)
        nw_row = sing.tile([P, NWT], f32)      # ||w_row||
        nc.scalar.activation(nw_row[:], nwsq[:], AF.Sqrt)
        inw_row = sing.tile([P, NWT], f32)
        inw_scr = sing.tile([P, NWT], f32)
        nc.vector.reciprocal_approx_accurate(inw_row[:], nw_row[:], inw_scr[:])

        # normalized anchors (one wide mul with broadcast scale), transposed
        Vn = sing.tile([P, NWT, D], f32)
        VT = sing.tile([P, CS], f32r)
        inw_b = inw_row[:, :, None].to_broadcast([P, NWT, D])
        nc.vector.tensor_mul(Vn[:], Wn[:], inw_b)
        for g in range(4):
            pst = ps_tr.tile([P, 4 * P], f32, tag="tr")
            for t in range(4):
                nc.tensor.transpose(pst[:, t * P : (t + 1) * P],
                                    Vn[:, 4 * g + t], ident[:])
            nc.scalar.copy(VT[:, g * 512 : (g + 1) * 512], pst[:])

        # selector matrix per pair: cols 0:32 = ||w||*onehot, 32:64 = onehot
        # nonzero col for (pair q, parity) is 2q+par -> flat idx 66q + par
        selb = sing.tile([P, NPAIR, 2 * C_LOC], bf16)
        nc.vector.memset(selb[:], 0.0)
        self_flat = selb[:].rearrange("p q c -> p (q c)")
        for par in range(2):
            psl = slice(par * 64, par * 64 + 64)
            a0 = self_flat[psl, par : par + 1]
            dst_nw = bass.AP(tensor=a0.tensor, offset=a0.offset,
                             ap=[a0.ap[0], [2 * C_LOC + 2, NPAIR]])
            nc.vector.tensor_copy(dst_nw, nw_row[psl, :])
            a1 = self_flat[psl, C_LOC + par : C_LOC + par + 1]
            dst_one = bass.AP(tensor=a1.tensor, offset=a1.offset,
                              ap=[a1.ap[0], [2 * C_LOC + 2, NPAIR]])
            nc.vector.memset(dst_one, 1.0)

        tiles = {}
        pb = {}

        def emit_iter(ba, bb, bc):
            """One pipeline iteration: stage-A chunks of tile ba interleaved
            with stage-B groups of tile bb, then stage-C of tile bc."""
            # --- allocations ---
            if ba is not None:
                u = work.tile([P, CS], f32, tag="u", bufs=3)
            if bb is not None:
                x0 = tiles.pop(bb)
                Et = work.tile([P, NPAIR, P], bf16, tag="Et", bufs=2)
                xs = work.tile([P, NPAIR, P], bf16, tag="xs", bufs=2)
                prods = work.tile([P, NPAIR, 2 * P], bf16, tag="prods", bufs=2)
                Et3 = Et[:]
                xs3 = xs[:]
                pr3 = prods[:]
                hps = ps_h.tile([P, CS], f32, tag="h")
                pb[bb] = prods
            # --- interleaved A-chunks and B transpose groups (4 each) ---
            for g in range(4):
                if ba is not None:
                    js = slice(g * 512, (g + 1) * 512)
                    dps = ps_a.tile([P, 512], f32, tag="mm")
                    nc.tensor.matmul(dps[:], embT[:, ba * P : (ba + 1) * P],
                                     VT[:, js])
                    nc.scalar.activation(u[:, js], dps[:], AF.Copy,
                                         bias=BIG, scale=hine[:, ba : ba + 1])
                if bb is not None:
                    qs = slice(4 * g, 4 * g + 4)
                    xps = ps_tr.tile([P, 512], f32, tag="tr")
                    for j in range(4):
                        q = 4 * g + j
                        nc.tensor.transpose(xps[:, j * P : (j + 1) * P],
                                            x0[:, q * P : (q + 1) * P],
                                            ident[:])
                    nc.scalar.activation(
                        Et3[:, qs, :].rearrange("p q x -> p (q x)"),
                        xps[:], AF.Exp, bias=negbig[:])
                    nc.scalar.activation(
                        xs3[:, qs, :].rearrange("p q x -> p (q x)"),
                        xps[:], AF.Copy, bias=-BIG)
            # --- B: prod_d on gpsimd (all-SBUF), H matmuls on PE ---
            if bb is not None:
                nc.gpsimd.tensor_mul(pr3[:, :, 0:P], xs3, Et3)
                for q in range(NPAIR):
                    nc.tensor.matmul(hps[:, q * P : (q + 1) * P],
                                     GPb[:, q, :], Et3[:, q, :])
            # --- A: selection ---
            if ba is not None:
                mx8 = small.tile([P, C_LOC * 8], f32, tag="mx8", bufs=2)
                for c in range(C_LOC):
                    nc.vector.max(out=mx8[:, c * 8 : (c + 1) * 8],
                                  in_=u[:, c * S : (c + 1) * S])
                mask = work.tile([P, CS], f32, tag="mask", bufs=2)
                u3 = u[:].rearrange("p (c s) -> p c s", s=S)
                m3 = mask[:].rearrange("p (c s) -> p c s", s=S)
                thr = mx8[:].rearrange("p (c k) -> p c k", k=8)[:, :, 7]
                thr_b = thr[:, :, None].to_broadcast([P, C_LOC, S])
                nc.vector.tensor_tensor(m3, u3, thr_b, ALU.is_ge)
                x0a = work.tile([P, CS], f32, tag="x0", bufs=3)
                nc.gpsimd.tensor_mul(x0a[:], u[:], mask[:])
                tiles[ba] = x0a
            # --- B: prod_n ---
            if bb is not None:
                h3 = hps[:].rearrange("p (q x) -> p q x", x=P)
                nc.vector.tensor_mul(pr3[:, :, P : 2 * P], Et3, h3)
            # --- C: selector reduction + evac ---
            if bc is not None:
                prc = pb.pop(bc)[:]
                selps = ps_sel.tile([64, 2 * P], f32, tag="sel")
                for q in range(NPAIR):
                    nc.tensor.matmul(selps[:], selb[:, q, :], prc[:, q, :],
                                     start=(q == 0), stop=(q == NPAIR - 1))
                nc.scalar.copy(dznp[:, bc, :], selps[:])

        # per-class raw Gram matrices, packed block-diagonal per pair
        def build_gram():
            WTb = sing.tile([P, CS], bf16)
            for g in range(4):
                pst = ps_tr.tile([P, 512], f32, tag="tr")
                for t in range(4):
                    nc.tensor.transpose(pst[:, t * P : (t + 1) * P],
                                        Wn[:, 4 * g + t], ident[:])
                nc.scalar.copy(WTb[:, g * 512 : (g + 1) * 512], pst[:])
            Gt = sing.tile([S, CS], bf16)
            for g in range(4):
                psg = ps_h.tile([P, 512], f32, tag="h")
                for i in range(8):
                    c = 8 * g + i
                    cs = slice(c * S, (c + 1) * S)
                    nc.tensor.matmul(psg[0:S, i * S : (i + 1) * S],
                                     WTb[:, cs], WTb[:, cs])
                nc.scalar.copy(Gt[:, g * 512 : (g + 1) * 512], psg[0:S, :])
            GP = sing.tile([P, NPAIR, P], bf16)
            nc.vector.memset(GP[:], 0.0)
            gt3 = Gt[:].rearrange("p (q j) -> p q j", j=2 * S)
            gp3 = GP[:]
            nc.sync.dma_start(gp3[0:S, :, 0:S], gt3[:, :, 0:S])
            nc.sync.dma_start(gp3[S : 2 * S, :, S : 2 * S], gt3[:, :, S : 2 * S])
            return GP

        dznp = sing.tile([64, NBT, 2 * P], f32)

        # warm the Exp table before the steady loop (sqrt uses are done)
        texp = sing.tile([P, 1], f32)
        nc.scalar.activation(texp[:], bigt[:], AF.Exp)

        # ---------------- software-pipelined main loop ----------------
        emit_iter(0, None, None)
        emit_iter(1, None, None)
        GPb = build_gram()
        for k in range(2, NBT):
            emit_iter(k, k - 2, k - 3 if k >= 3 else None)
        emit_iter(None, NBT - 2, NBT - 3)
        emit_iter(None, NBT - 1, NBT - 2)
        emit_iter(None, None, NBT - 1)

        # ---------------- tail ----------------
        # out = 10 * (h*d2z) / sqrt(np2z) + OUT_BIAS
        d2zv = dznp[0:C_LOC, :, 0:P]                       # [32, 8, 128]
        np2v = dznp[C_LOC : 2 * C_LOC, :, P : 2 * P]       # [32, 8, 128]
        nps = sing.tile([C_LOC, NBT, P], f32)
        nc.scalar.activation(nps[:], np2v, AF.Sqrt, scale=0.01)
        rs = sing.tile([C_LOC, NBT, P], f32)
        t1 = sing.tile([C_LOC, NBT, P], f32)
        nc.vector.reciprocal_approx_accurate(rs[:], nps[:], t1[:])
        nc.vector.tensor_mul(t1[:], d2zv, rs[:])
        osb = sing.tile([P, NBT, C_LOC], f32)
        for g in range(2):
            pst = ps_tr.tile([P, 512], f32, tag="tr")
            for t in range(4):
                bt = 4 * g + t
                nc.tensor.transpose(pst[:, t * C_LOC : (t + 1) * C_LOC],
                                    t1[:, bt, :], ident[0:C_LOC, 0:C_LOC])
            nc.scalar.activation(
                osb[:, 4 * g : 4 * g + 4, :].rearrange("p t c -> p (t c)"),
                pst[:, 0 : 4 * C_LOC], AF.Copy, bias=OUT_BIAS)
        nc.sync.dma_start(out_d[:].rearrange("(t p) c -> p t c", p=P), osb[:])

    nc.compile()
    return nc


def _get_nc():
    if "nc" not in _CACHE:
        _CACHE["nc"] = build_nc()
    return _CACHE["nc"]


def kernel(emb: np.ndarray, weight: np.ndarray) -> np.ndarray:
    from concourse.bass_utils import run_bass_kernel_spmd

    emb = np.ascontiguousarray(np.asarray(emb, dtype=np.float32))
    weight = np.ascontiguousarray(np.asarray(weight, dtype=np.float32))
    assert emb.shape == (B, D) and weight.shape == (C, S, D)

    nc = _get_nc()
    in_maps = [
        {
            "emb": emb,
            "weight": np.ascontiguousarray(
                weight[i * C_LOC : (i + 1) * C_LOC].reshape(CS, D)
            ),
        }
        for i in range(NCORES)
    ]
    res = run_bass_kernel_spmd(nc, in_maps, core_ids=list(range(NCORES)))
    return np.concatenate(
        [res.results[i]["out"] for i in range(NCORES)], axis=1
    )



# revision 23
# speedup vs baseline: 1.5617x; 1.0115x over previous
"""Trainium2 Bass kernel for the moe_routing classifier problem (v2).

Computation (per batch row b, class c):
  cos[b,c,s]  = cosine(emb[b], weight[c,s])            (64 sub-prototypes)
  top-8 over s, softmax weights w, protos = sum_k w_k * weight[c, idx_k]
  out[b,c]    = ((1 + cosine(protos, emb[b])) / 2 + 1e-8) / 0.1

Key algebra (per (b,c); E = masked exp of scores, any per-(b,c) scale of E
cancels between numerator and denominator):
  u          = 0.5/||emb_b|| * dotn + BIG   (dotn = emb . What, normalized W)
  x          = u * 1[u >= thr8]             (thr8 = 8th largest u per class)
  Et         = exp(x^T - BIG)               (unselected -> e^-BIG ~ 1e-13)
  h*d2z      = sum_s ||w_s|| * Et * (x^T - BIG)
  np2z       = Et^T G Et  via pair-block-diag Gram matmuls
  out        = 10 * (h*d2z) / sqrt(np2z) + 5 + 1e-7   (all ||emb|| cancel)

Both reductions over s run on the PE array: one accumulating matmul per
class-pair against a [128, 64] selector whose cols 0:32 hold ||w_s||*onehot
(for d2z) and cols 32:64 hold onehot (for np2z).

Sharding: classes split across 8 cores (32 each), emb replicated; each core
writes a [1024, 32] slice of the output.
"""

import numpy as np

B, D, C, S = 1024, 128, 256, 64
NCORES = 8
C_LOC = C // NCORES        # 32 classes per core
CS = C_LOC * S             # 2048 anchor rows per core
P = 128                    # partitions
NBT = B // P               # 8 batch tiles
NWT = CS // P              # 16 weight tiles (= class pairs)
NPAIR = NWT
BIG = 30.0                 # separation constant for masked exp
OUT_BIAS = 5.0 + 1e-7      # ((1+x)/2 + 1e-8) / 0.1 = 5x + 5 + 1e-7

_CACHE = {}


def build_nc():
    import concourse.bass as bass
    import concourse.tile as tile
    from concourse import bacc, mybir
    from concourse.masks import make_identity
    from contextlib import ExitStack

    f32 = mybir.dt.float32
    f32r = mybir.dt.float32r
    bf16 = mybir.dt.bfloat16
    AF = mybir.ActivationFunctionType
    ALU = mybir.AluOpType

    nc = bacc.Bacc(None, target_bir_lowering=False)
    emb_d = nc.dram_tensor("emb", [B, D], f32, kind="ExternalInput")
    w_d = nc.dram_tensor("weight", [CS, D], f32, kind="ExternalInput")
    out_d = nc.dram_tensor("out", [B, C_LOC], f32, kind="ExternalOutput")

    with tile.TileContext(nc) as tc, ExitStack() as ctx:
        sing = ctx.enter_context(tc.tile_pool(name="sing", bufs=1))
        work = ctx.enter_context(tc.tile_pool(name="work", bufs=2))
        small = ctx.enter_context(tc.tile_pool(name="small", bufs=2))
        ps_a = ctx.enter_context(tc.tile_pool(name="ps_a", bufs=1, space="PSUM"))
        ps_tr = ctx.enter_context(tc.tile_pool(name="ps_tr", bufs=2, space="PSUM"))
        ps_h = ctx.enter_context(tc.tile_pool(name="ps_h", bufs=1, space="PSUM"))
        ps_sel = ctx.enter_context(tc.tile_pool(name="ps_sel", bufs=1, space="PSUM"))

        ident = sing.tile([P, P], f32)
        make_identity(nc, ident[:])
        negbig = sing.tile([P, 1], f32)
        nc.vector.memset(negbig[:], -BIG)
        bigt = sing.tile([P, 1], f32)
        nc.vector.memset(bigt[:], BIG)

        # ---------------- load inputs (emb first, separate DMA queues) ----
        En = sing.tile([P, NBT, D], f32)
        nc.sync.dma_start(En[:], emb_d[:].rearrange("(t p) d -> p t d", p=P))
        Wn = sing.tile([P, NWT, D], f32)
        nc.gpsimd.dma_start(Wn[:], w_d[:].rearrange("(t p) d -> p t d", p=P))

        # ---------------- emb-side preproc ----------------
        # squares on DVE (mult+reduce) to keep the scalar engine free
        esqt = sing.tile([P, B], f32)
        nc.gpsimd.tensor_mul(esqt[:], En[:].rearrange("p t d -> p (t d)"),
                             En[:].rearrange("p t d -> p (t d)"))
        esq = sing.tile([P, NBT], f32)
        nc.vector.tensor_reduce(esq[:], esqt[:].rearrange("p (t d) -> p t d", d=D),
                                axis=mybir.AxisListType.X, op=ALU.add)
        ne = sing.tile([P, NBT], f32)          # ||emb||  (sqrt table set)
        nc.scalar.activation(ne[:], esq[:], AF.Sqrt)
        ine = sing.tile([P, NBT], f32)
        hine = sing.tile([P, NBT], f32)        # 0.5/||emb||
        nc.vector.reciprocal_approx_accurate(ine[:], ne[:], hine[:])
        nc.vector.tensor_scalar_mul(hine[:], ine[:], 0.5)

        embT = sing.tile([P, B], f32r)         # emb^T [d, b]
        for g in range(2):
            pst = ps_tr.tile([P, 4 * P], f32, tag="tr")
            for t in range(4):
                nc.tensor.transpose(pst[:, t * P : (t + 1) * P],
                                    En[:, 4 * g + t], ident[:])
            nc.scalar.copy(embT[:, g * 512 : (g + 1) * 512], pst[:])

        # ---------------- weight-side preproc ----------------
        wsqt = sing.tile([P, CS], f32)
        nc.gpsimd.tensor_mul(wsqt[:], Wn[:].rearrange("p t d -> p (t d)"),
                             Wn[:].rearrange("p t d -> p (t d)"))
        nwsq = sing.tile([P, NWT], f32)        # ||w_row||^2 row-tiled
        nc.vector.tensor_reduce(nwsq[:], wsqt[:].rearrange("p (t d) -> p t d", d=D),
                                axis=mybir.AxisListType.X, op=ALU.add)
        nw_row = sing.tile([P, NWT], f32)      # ||w_row||
        nc.scalar.activation(nw_row[:], nwsq[:], AF.Sqrt)
        inw_row = sing.tile([P, NWT], f32)
        inw_scr = sing.tile([P, NWT], f32)
        nc.vector.reciprocal_approx_accurate(inw_row[:], nw_row[:], inw_scr[:])

        # normalized anchors (one wide mul with broadcast scale), transposed
        Vn = sing.tile([P, NWT, D], f32)
        VT = sing.tile([P, CS], f32r)
        inw_b = inw_row[:, :, None].to_broadcast([P, NWT, D])
        nc.vector.tensor_mul(Vn[:], Wn[:], inw_b)
        for g in range(4):
            pst = ps_tr.tile([P, 4 * P], f32, tag="tr")
            for t in range(4):
                nc.tensor.transpose(pst[:, t * P : (t + 1) * P],
                                    Vn[:, 4 * g + t], ident[:])
            nc.scalar.copy(VT[:, g * 512 : (g + 1) * 512], pst[:])

        # selector matrix per pair: cols 0:32 = ||w||*onehot, 32:64 = onehot
        # nonzero col for (pair q, parity) is 2q+par -> flat idx 66q + par
        selb = sing.tile([P, NPAIR, 2 * C_LOC], bf16)
        nc.gpsimd.memset(selb[:], 0.0)
        self_flat = selb[:].rearrange("p q c -> p (q c)")
        for par in range(2):
            psl = slice(par * 64, par * 64 + 64)
            a0 = self_flat[psl, par : par + 1]
            dst_nw = bass.AP(tensor=a0.tensor, offset=a0.offset,
                             ap=[a0.ap[0], [2 * C_LOC + 2, NPAIR]])
            nc.vector.tensor_copy(dst_nw, nw_row[psl, :])
            a1 = self_flat[psl, C_LOC + par : C_LOC + par + 1]
            dst_one = bass.AP(tensor=a1.tensor, offset=a1.offset,
                              ap=[a1.ap[0], [2 * C_LOC + 2, NPAIR]])
            nc.vector.memset(dst_one, 1.0)

        tiles = {}
        pb = {}

        def emit_iter(ba, bb, bc):
            """One pipeline iteration: stage-A chunks of tile ba interleaved
            with stage-B groups of tile bb, then stage-C of tile bc."""
            # --- allocations ---
            if ba is not None:
                u = work.tile([P, CS], f32, tag="u", bufs=3)
            if bb is not None:
                x0 = tiles.pop(bb)
                Et = work.tile([P, NPAIR, P], bf16, tag="Et", bufs=2)
                xs = work.tile([P, NPAIR, P], bf16, tag="xs", bufs=2)
                prods = work.tile([P, NPAIR, 2 * P], bf16, tag="prods", bufs=2)
                Et3 = Et[:]
                xs3 = xs[:]
                pr3 = prods[:]
                hps = ps_h.tile([P, CS], f32, tag="h")
                pb[bb] = prods
            # --- interleaved A-chunks and B transpose groups (4 each) ---
            for g in range(4):
                if ba is not None:
                    js = slice(g * 512, (g + 1) * 512)
                    dps = ps_a.tile([P, 512], f32, tag="mm")
                    nc.tensor.matmul(dps[:], embT[:, ba * P : (ba + 1) * P],
                                     VT[:, js])
                    nc.scalar.activation(u[:, js], dps[:], AF.Copy,
                                         bias=BIG, scale=hine[:, ba : ba + 1])
                if bb is not None:
                    qs = slice(4 * g, 4 * g + 4)
                    xps = ps_tr.tile([P, 512], f32, tag="tr")
                    for j in range(4):
                        q = 4 * g + j
                        nc.tensor.transpose(xps[:, j * P : (j + 1) * P],
                                            x0[:, q * P : (q + 1) * P],
                                            ident[:])
                    nc.scalar.activation(
                        Et3[:, qs, :].rearrange("p q x -> p (q x)"),
                        xps[:], AF.Exp, bias=negbig[:])
                    nc.scalar.activation(
                        xs3[:, qs, :].rearrange("p q x -> p (q x)"),
                        xps[:], AF.Copy, bias=-BIG)
            # --- B: prod_d on gpsimd (all-SBUF), H matmuls on PE ---
            if bb is not None:
                nc.gpsimd.tensor_mul(pr3[:, :, 0:P], xs3, Et3)
                for q in range(NPAIR):
                    nc.tensor.matmul(hps[:, q * P : (q + 1) * P],
                                     GPb[:, q, :], Et3[:, q, :])
            # --- A: selection ---
            if ba is not None:
                mx8 = small.tile([P, C_LOC * 8], f32, tag="mx8", bufs=2)
                for c in range(C_LOC):
                    nc.vector.max(out=mx8[:, c * 8 : (c + 1) * 8],
                                  in_=u[:, c * S : (c + 1) * S])
                mask = work.tile([P, CS], bf16, tag="mask", bufs=2)
                u3 = u[:].rearrange("p (c s) -> p c s", s=S)
                m3 = mask[:].rearrange("p (c s) -> p c s", s=S)
                thr = mx8[:].rearrange("p (c k) -> p c k", k=8)[:, :, 7]
                thr_b = thr[:, :, None].to_broadcast([P, C_LOC, S])
                nc.vector.tensor_tensor(m3, u3, thr_b, ALU.is_ge)
                x0a = work.tile([P, CS], f32, tag="x0", bufs=3)
                nc.gpsimd.tensor_mul(x0a[:], u[:], mask[:])
                tiles[ba] = x0a
            # --- B: prod_n ---
            if bb is not None:
                h3 = hps[:].rearrange("p (q x) -> p q x", x=P)
                nc.vector.tensor_mul(pr3[:, :, P : 2 * P], Et3, h3)
            # --- C: selector reduction + evac ---
            if bc is not None:
                prc = pb.pop(bc)[:]
                selps = ps_sel.tile([64, 2 * P], f32, tag="sel")
                for q in range(NPAIR):
                    nc.tensor.matmul(selps[:], selb[:, q, :], prc[:, q, :],
                                     start=(q == 0), stop=(q == NPAIR - 1))
                nc.scalar.copy(dznp[:, bc, :], selps[:])

        # per-class raw Gram matrices, packed block-diagonal per pair
        def build_gram():
            WTb = sing.tile([P, CS], bf16)
            for g in range(4):
                pst = ps_tr.tile([P, 512], f32, tag="tr")
                for t in range(4):
                    nc.tensor.transpose(pst[:, t * P : (t + 1) * P],
                                        Wn[:, 4 * g + t], ident[:])
                nc.scalar.copy(WTb[:, g * 512 : (g + 1) * 512], pst[:])
            Gt = sing.tile([S, CS], bf16)
            for g in range(4):
                psg = ps_h.tile([P, 512], f32, tag="h")
                for i in range(8):
                    c = 8 * g + i
                    cs = slice(c * S, (c + 1) * S)
                    nc.tensor.matmul(psg[0:S, i * S : (i + 1) * S],
                                     WTb[:, cs], WTb[:, cs])
                nc.scalar.copy(Gt[:, g * 512 : (g + 1) * 512], psg[0:S, :])
            GP = sing.tile([P, NPAIR, P], bf16)
            nc.gpsimd.memset(GP[:], 0.0)
            gt3 = Gt[:].rearrange("p (q j) -> p q j", j=2 * S)
            gp3 = GP[:]
            nc.sync.dma_start(gp3[0:S, :, 0:S], gt3[:, :, 0:S])
            nc.sync.dma_start(gp3[S : 2 * S, :, S : 2 * S], gt3[:, :, S : 2 * S])
            return GP

        dznp = sing.tile([64, NBT, 2 * P], f32)

        # warm the Exp table before the steady loop (sqrt uses are done)
        texp = sing.tile([P, 1], f32)
        nc.scalar.activation(texp[:], bigt[:], AF.Exp)

        # ---------------- software-pipelined main loop ----------------
        emit_iter(0, None, None)
        emit_iter(1, None, None)
        GPb = build_gram()
        for k in range(2, NBT):
            emit_iter(k, k - 2, k - 3 if k >= 3 else None)
        emit_iter(None, NBT - 2, NBT - 3)
        emit_iter(None, NBT - 1, NBT - 2)
        emit_iter(None, None, NBT - 1)

        # ---------------- tail ----------------
        # out = 10 * (h*d2z) / sqrt(np2z) + OUT_BIAS
        d2zv = dznp[0:C_LOC, :, 0:P]                       # [32, 8, 128]
        np2v = dznp[C_LOC : 2 * C_LOC, :, P : 2 * P]       # [32, 8, 128]
        nps = sing.tile([C_LOC, NBT, P], f32)
        nc.scalar.activation(nps[:], np2v, AF.Sqrt, scale=0.01)
        rs = sing.tile([C_LOC, NBT, P], f32)
        t1 = sing.tile([C_LOC, NBT, P], f32)
        nc.vector.reciprocal_approx_accurate(rs[:], nps[:], t1[:])
        nc.vector.tensor_mul(t1[:], d2zv, rs[:])
        osb = sing.tile([P, NBT, C_LOC], f32)
        for g in range(2):
            pst = ps_tr.tile([P, 512], f32, tag="tr")
            for t in range(4):
                bt = 4 * g + t
                nc.tensor.transpose(pst[:, t * C_LOC : (t + 1) * C_LOC],
                                    t1[:, bt, :], ident[0:C_LOC, 0:C_LOC])
            nc.scalar.activation(
                osb[:, 4 * g : 4 * g + 4, :].rearrange("p t c -> p (t c)"),
                pst[:, 0 : 4 * C_LOC], AF.Copy, bias=OUT_BIAS)
        nc.sync.dma_start(out_d[:].rearrange("(t p) c -> p t c", p=P), osb[:])

    nc.compile()
    return nc


def _get_nc():
    if "nc" not in _CACHE:
        _CACHE["nc"] = build_nc()
    return _CACHE["nc"]


def kernel(emb: np.ndarray, weight: np.ndarray) -> np.ndarray:
    from concourse.bass_utils import run_bass_kernel_spmd

    emb = np.ascontiguousarray(np.asarray(emb, dtype=np.float32))
    weight = np.ascontiguousarray(np.asarray(weight, dtype=np.float32))
    assert emb.shape == (B, D) and weight.shape == (C, S, D)

    nc = _get_nc()
    in_maps = [
        {
            "emb": emb,
            "weight": np.ascontiguousarray(
                weight[i * C_LOC : (i + 1) * C_LOC].reshape(CS, D)
            ),
        }
        for i in range(NCORES)
    ]
    res = run_bass_kernel_spmd(nc, in_maps, core_ids=list(range(NCORES)))
    return np.concatenate(
        [res.results[i]["out"] for i in range(NCORES)], axis=1
    )

